# revision 6
# baseline (speedup 1.0000x reference)
"""CoLightNet Trainium2 Bass kernel (self-contained).

SPMD over 8 cores; core c owns output rows [c*1024, (c+1)*1024).
  inputs : stateT   [S,N]    bf16 (replicated, host-transposed state)
           smbT     [S,Mc]   bf16 (core's own row block, transposed)
           adjt     [N,Mc]   i8   (transposed adjacency: adj[rows].T)
           w1,w2,wq,wk,wh1 [128,128], wh2 [128,8], biases as [*,1] f32
           (wq is pre-scaled by 1/sqrt(E) on the host)
  output : outb     [Mc,A]   f32

Math (identical to the reference, reformulated):
  h    = relu(state@w1+b1)@w2+b2
  sT   = (h@wk)^T-stationary x (h@wq/sqrt(E))-moving       # scores^T [n,m]
  wT   = exp(sT) * adj^T                                   # exact masking
  aggT = h^T-stationary x wT-moving  (per n-block, PSUM-accumulated)
  den  = column-sum of wT (GpSimd-accumulated, PE-reduced)
  head : relu(agg/den @ wh1 + bh1) @ wh2 + bh2
       = [relu(agg@wh1 + bh1 (x) den) / den] @ wh2 + bh2   # rank-1 bias trick

MLP/attention-weight matmuls run bf16 (moving dim 512); the q/k
projections stay float32r so the pre-exp scores keep fp32-level accuracy.
"""

from contextlib import ExitStack

import concourse.bass as bass
import concourse.mybir as mybir
import concourse.tile as tile
from concourse import bacc
from concourse.masks import make_identity

F32 = mybir.dt.float32
F32R = mybir.dt.float32r
BF16 = mybir.dt.bfloat16
I8 = mybir.dt.int8
AF = mybir.ActivationFunctionType
ALU = mybir.AluOpType

S = 128
E = 128
A = 8


def ts(i, size):
    return slice(i * size, (i + 1) * size)


def build_kernel(n_total=8192, m_core=1024):
    nc = bacc.Bacc("TRN2", debug=False)
    stateT = nc.dram_tensor("stateT", (S, n_total), BF16, kind="ExternalInput").ap()
    smbT = nc.dram_tensor("smbT", (S, m_core), BF16, kind="ExternalInput").ap()
    adjt = nc.dram_tensor("adjt", (n_total, m_core), I8, kind="ExternalInput").ap()
    wt = {}
    for name, shape in [
        ("w1", (S, E)), ("w2", (E, E)), ("wq", (E, E)), ("wk", (E, E)),
        ("wh1", (E, E)), ("wh2", (E, A)),
        ("b1", (E, 1)), ("b2", (E, 1)), ("bh1", (E, 1)), ("bh2", (A, 1)),
    ]:
        wt[name] = nc.dram_tensor(name, shape, F32, kind="ExternalInput").ap()
    outb = nc.dram_tensor("outb", (m_core, A), F32, kind="ExternalOutput").ap()

    with tile.TileContext(nc) as tc:
        colight_body(tc, outb, stateT, smbT, adjt, wt)
    nc.compile()
    return nc


def colight_body(tc, outb, stateT, smbT, adjt, wt):
    nc = tc.nc
    n_total = stateT.shape[1]
    m_core = adjt.shape[1]
    NT = n_total // 512   # n-groups of 512
    NB = n_total // 128   # n-blocks of 128
    MB = m_core // 128    # m-blocks of 128

    with ExitStack() as ctx:
        singles = ctx.enter_context(tc.tile_pool(name="singles", bufs=1))

        # ---- constants ----
        wf = {}
        for name, shape in [("w1", [S, E]), ("w2", [E, E]), ("wq", [E, E]),
                            ("wk", [E, E]), ("wh1", [E, E]), ("wh2", [E, A]),
                            ("b1", [E, 1]), ("b2", [E, 1]), ("bh1", [E, 1]),
                            ("bh2", [A, 1])]:
            t = singles.tile(shape, F32, tag=f"w_{name}")
            nc.scalar.dma_start(out=t, in_=wt[name])
            wf[name] = t
        w1b = singles.tile([S, E], BF16)
        w2b = singles.tile([E, E], BF16)
        wh1b = singles.tile([E, E], BF16)
        wh2b = singles.tile([E, A], BF16)
        wqr = singles.tile([E, E], F32R)
        wkr = singles.tile([E, E], F32R)
        for dst, src in [(w1b, "w1"), (w2b, "w2"), (wh1b, "wh1"), (wh2b, "wh2"),
                         (wqr, "wq"), (wkr, "wk")]:
            nc.vector.tensor_copy(out=dst, in_=wf[src])
        # bh1 as a 1-partition row (rank-1 bias update), bh2 broadcast tile
        bh1_row = singles.tile([1, E], F32)
        nc.scalar.dma_start(out=bh1_row, in_=wt["bh1"].rearrange("e o -> o e"))
        bh1_row_b = singles.tile([1, E], BF16)
        nc.vector.tensor_copy(out=bh1_row_b, in_=bh1_row)
        bh2_bcast = singles.tile([128, A], F32)
        nc.scalar.dma_start(
            out=bh2_bcast,
            in_=wt["bh2"].rearrange("a o -> o a").to_broadcast([128, A]),
        )
        ident = singles.tile([128, 128], F32)
        make_identity(nc, ident)
        identr = singles.tile([128, 128], F32R)
        nc.vector.tensor_copy(out=identr, in_=ident)
        ones_f = singles.tile([128, 1], F32)
        nc.vector.memset(ones_f, 1.0)
        ones_r = singles.tile([128, 1], F32R)
        nc.vector.tensor_copy(out=ones_r, in_=ones_f)

        # ---- persistent activations ----
        stT = singles.tile([S, n_total], BF16)
        nc.sync.dma_start(out=stT, in_=stateT)
        smbT_t = singles.tile([S, m_core], BF16)
        nc.sync.dma_start(out=smbT_t, in_=smbT)
        h1T = singles.tile([128, n_total], BF16)
        hT = singles.tile([128, n_total], F32R)
        kTs = singles.tile([E, n_total], F32R)
        qTs = singles.tile([E, m_core], F32R)
        hblk = singles.tile([128, NB, E], BF16)   # h in normal orientation
        den_acc = singles.tile([128, m_core], F32)
        agg_sb = singles.tile([128, m_core], BF16)
        den_row = singles.tile([1, m_core], F32)
        den_row_b = singles.tile([1, m_core], BF16)

        nc.gpsimd.memset(den_acc, 0.0)

        ph1_stack = ExitStack()
        ph1_sb = ph1_stack.enter_context(tc.tile_pool(name="ph1_sb", bufs=3))
        ph1_ps = ph1_stack.enter_context(tc.tile_pool(name="ph1_ps", bufs=4, space="PSUM"))

        # ---- q-path first (phase 2 needs qTs for every block) ----
        for j in range(m_core // 512):
            ps1 = ph1_ps.tile([128, 512], F32, tag="p")
            nc.tensor.matmul(ps1, w1b, smbT_t[:, ts(j, 512)], start=True, stop=True)
            h1m = ph1_sb.tile([128, 512], BF16, tag="sbq")
            nc.scalar.activation(h1m, ps1, AF.Relu, bias=wf["b1"], scale=1.0)
            ps2 = ph1_ps.tile([128, 512], F32, tag="p")
            nc.tensor.matmul(ps2, w2b, h1m, start=True, stop=True)
            hm = ph1_sb.tile([128, 512], F32R, tag="sbq")
            nc.scalar.activation(hm, ps2, AF.Identity, bias=wf["b2"], scale=1.0)
            ps3 = ph1_ps.tile([128, 512], F32, tag="p")
            nc.tensor.matmul(ps3, wqr, hm, start=True, stop=True)
            nc.scalar.copy(out=qTs[:, ts(j, 512)], in_=ps3)

        # ---- fused per-512 pipeline: h1T -> hT -> kT -> h transposes ----
        for j in range(NT):
            ps = ph1_ps.tile([128, 512], F32, tag="p")
            nc.tensor.matmul(ps, w1b, stT[:, ts(j, 512)], start=True, stop=True)
            nc.scalar.activation(h1T[:, ts(j, 512)], ps, AF.Relu, bias=wf["b1"], scale=1.0)
            ps2 = ph1_ps.tile([128, 512], F32, tag="p")
            nc.tensor.matmul(ps2, w2b, h1T[:, ts(j, 512)], start=True, stop=True)
            nc.scalar.activation(hT[:, ts(j, 512)], ps2, AF.Identity, bias=wf["b2"], scale=1.0)
            ps3 = ph1_ps.tile([128, 512], F32, tag="p")
            nc.tensor.matmul(ps3, wkr, hT[:, ts(j, 512)], start=True, stop=True)
            nc.vector.tensor_copy(out=kTs[:, ts(j, 512)], in_=ps3)
            hx = ph1_ps.tile([128, 512], F32R, tag="p")
            for a in range(4):
                nb = j * 4 + a
                nc.tensor.transpose(hx[:, ts(a, 128)], hT[:, ts(nb, 128)], identr)
            nc.vector.tensor_copy(
                out=hblk[:, ts(j, 4), :],
                in_=hx.rearrange("p (a e) -> p a e", a=4),
            )

        ph1_stack.close()

        # ---- phase 2: transposed masked attention over both m-halves ----
        ph2_stack = ExitStack()
        adj_pool = ph2_stack.enter_context(tc.tile_pool(name="adj", bufs=3))
        exp_pool = ph2_stack.enter_context(tc.tile_pool(name="expT", bufs=8))
        sc_ps = ph2_stack.enter_context(tc.tile_pool(name="sc_ps", bufs=3, space="PSUM"))
        agg_psp = ph2_stack.enter_context(tc.tile_pool(name="agg_ps", bufs=1, space="PSUM"))

        agg0 = agg_psp.tile([128, 512], F32, tag="agg0")
        agg1 = agg_psp.tile([128, 512], F32, tag="agg1")

        for ng in range(NT):
            adjt_t = adj_pool.tile([128, 4, m_core], I8)
            nc.sync.dma_start(
                out=adjt_t,
                in_=adjt[ts(ng, 512), :].rearrange("(a p) m -> p a m", p=128),
            )
            for a in range(4):
                nb = ng * 4 + a
                scp = sc_ps.tile([128, m_core], F32)
                nc.tensor.matmul(
                    scp[:, 0:512], kTs[:, ts(nb, 128)], qTs[:, 0:512],
                    start=True, stop=True,
                )
                nc.tensor.matmul(
                    scp[:, 512:1024], kTs[:, ts(nb, 128)], qTs[:, 512:1024],
                    start=True, stop=True,
                )
                exps = exp_pool.tile([128, m_core], BF16, tag="exps")
                nc.scalar.activation(exps, scp, AF.Exp, bias=0.0, scale=1.0)
                expT = exp_pool.tile([128, m_core], BF16, tag="expm")
                nc.vector.tensor_tensor(
                    out=expT, in0=exps, in1=adjt_t[:, a, :], op=ALU.mult,
                )
                nc.tensor.matmul(
                    agg0, hblk[:, nb, :], expT[:, 0:512],
                    start=(nb == 0), stop=(nb == NB - 1),
                )
                nc.tensor.matmul(
                    agg1, hblk[:, nb, :], expT[:, 512:1024],
                    start=(nb == 0), stop=(nb == NB - 1),
                )
                nc.gpsimd.tensor_tensor(
                    out=den_acc, in0=den_acc, in1=expT, op=ALU.add,
                )

        # aggT / den out of PSUM
        nc.vector.tensor_copy(out=agg_sb[:, 0:512], in_=agg0)
        nc.vector.tensor_copy(out=agg_sb[:, 512:1024], in_=agg1)
        den_acc_r = singles.tile([128, m_core], F32R)
        nc.vector.tensor_copy(out=den_acc_r, in_=den_acc)
        ph2_stack.close()

        # ---- head ----
        head_sb = ctx.enter_context(tc.tile_pool(name="head_sb", bufs=3))
        head_ps = ctx.enter_context(tc.tile_pool(name="head_ps", bufs=2, space="PSUM"))
        head_ps2 = ctx.enter_context(tc.tile_pool(name="head_ps2", bufs=1, space="PSUM"))

        for half in range(2):
            dps = head_ps2.tile([1, 512], F32, tag="dps")
            nc.tensor.matmul(dps, ones_r, den_acc_r[:, ts(half, 512)],
                             start=True, stop=True)
            nc.vector.tensor_copy(out=den_row[0:1, ts(half, 512)], in_=dps)
            nc.vector.tensor_copy(out=den_row_b[0:1, ts(half, 512)], in_=dps)

        for mb in range(MB):
            # h3T_unnorm = wh1^T@aggT + bh1 (x) den   (PSUM-accumulated)
            h3_ps = head_ps.tile([128, 128], F32, tag="hps")
            nc.tensor.matmul(h3_ps, wh1b, agg_sb[:, ts(mb, 128)],
                             start=True, stop=False)
            nc.tensor.matmul(h3_ps, bh1_row_b, den_row_b[0:1, ts(mb, 128)],
                             start=False, stop=True)
            h3 = head_sb.tile([128, 128], BF16, tag="hsb")
            nc.scalar.activation(h3, h3_ps, AF.Relu, bias=0.0, scale=1.0)
            oT_ps = head_ps.tile([8, 128], F32, tag="ops")
            nc.tensor.matmul(oT_ps, wh2b, h3, start=True, stop=True)
            oT = head_sb.tile([8, 128], F32, tag="osb")
            nc.vector.tensor_copy(out=oT, in_=oT_ps)
            # per-m reciprocal of den
            denT_ps = head_ps2.tile([128, 1], F32, tag="dtp")
            nc.tensor.transpose(denT_ps, den_row[0:1, ts(mb, 128)], ident[0:1, 0:1])
            rden = head_sb.tile([128, 1], F32, tag="rdn")
            nc.vector.reciprocal(rden, denT_ps)
            o_ps = head_ps2.tile([128, A], F32, tag="otp")
            nc.tensor.transpose(o_ps, oT, ident[0:8, 0:8])
            o_sb = head_sb.tile([128, A], F32, tag="fin")
            nc.vector.scalar_tensor_tensor(
                out=o_sb, in0=o_ps, scalar=rden, in1=bh2_bcast,
                op0=ALU.mult, op1=ALU.add,
            )
            nc.scalar.dma_start(out=outb[ts(mb, 128), :], in_=o_sb)


# ----------------------------------------------------------------------------
# Host entry point: full inputs in, full output out. 8-way row sharding.
# ----------------------------------------------------------------------------
import numpy as np
import ml_dtypes

N_TOTAL = 8192
N_CORES = 8
M_CORE = N_TOTAL // N_CORES

_cached = {}


def _get_nc():
    if "nc" not in _cached:
        _cached["nc"] = build_kernel(n_total=N_TOTAL, m_core=M_CORE)
    return _cached["nc"]


def make_in_maps(state_matrix, adj, w1, b1, w2, b2, wq, wk, wh1, bh1, wh2, bh2):
    state_matrix = np.asarray(state_matrix, dtype=np.float32)
    stateT = np.ascontiguousarray(state_matrix.T).astype(ml_dtypes.bfloat16)
    adj = np.asarray(adj)
    f32 = lambda x: np.ascontiguousarray(np.asarray(x, dtype=np.float32))
    wq_scaled = f32(wq) / np.float32(np.sqrt(E))
    # transposed adjacency as int8: adjt_full[n, m] = adj[m, n]
    adjt_full = np.ascontiguousarray(adj.T.astype(np.int8))
    common = {
        "stateT": stateT,
        "w1": f32(w1), "w2": f32(w2), "wq": wq_scaled, "wk": f32(wk),
        "wh1": f32(wh1), "wh2": f32(wh2),
        "b1": f32(b1).reshape(E, 1), "b2": f32(b2).reshape(E, 1),
        "bh1": f32(bh1).reshape(E, 1), "bh2": f32(bh2).reshape(A, 1),
    }
    in_maps = []
    for c in range(N_CORES):
        rows = slice(c * M_CORE, (c + 1) * M_CORE)
        in_maps.append(
            dict(
                common,
                smbT=np.ascontiguousarray(stateT[:, rows]),
                adjt=np.ascontiguousarray(adjt_full[:, rows]),
            )
        )
    return in_maps


def kernel(state_matrix, adj, w1, b1, w2, b2, wq, wk, wh1, bh1, wh2, bh2):
    from concourse import bass_utils

    in_maps = make_in_maps(
        state_matrix, adj, w1, b1, w2, b2, wq, wk, wh1, bh1, wh2, bh2
    )
    res = bass_utils.run_bass_kernel_spmd(
        _get_nc(), in_maps, core_ids=list(range(N_CORES))
    )
    out = np.concatenate([r["outb"] for r in res.results], axis=0)
    return out.astype(np.float32)


# revision 7
# speedup vs baseline: 1.4254x; 1.4254x over previous
"""CoLightNet Trainium2 Bass kernel (self-contained).

SPMD over 8 cores; core c owns output rows [c*1024, (c+1)*1024).
  inputs : stateT   [S,N]    bf16 (replicated, host-transposed state)
           smbT     [S,Mc]   bf16 (core's own row block, transposed)
           adjt     [N,Mc]   i8   (transposed adjacency: adj[rows].T)
           w1,w2,wq,wk,wh1 [128,128], wh2 [128,8], biases as [*,1] f32
           (wq is pre-scaled by 1/sqrt(E) on the host)
  output : outb     [Mc,A]   f32

Math (identical to the reference, reformulated):
  h    = relu(state@w1+b1)@w2+b2
  sT   = (h@wk)^T-stationary x (h@wq/sqrt(E))-moving       # scores^T [n,m]
  wT   = exp(sT) * adj^T                                   # exact masking
  aggT = h^T-stationary x wT-moving  (per n-block, PSUM-accumulated)
  den  = column-sum of wT (GpSimd-accumulated, PE-reduced)
  head : relu(agg/den @ wh1 + bh1) @ wh2 + bh2
       = [relu(agg@wh1 + bh1 (x) den) / den] @ wh2 + bh2   # rank-1 bias trick

MLP/attention-weight matmuls run bf16 (moving dim 512); the q/k
projections stay float32r so the pre-exp scores keep fp32-level accuracy.
"""

from contextlib import ExitStack

import concourse.bass as bass
import concourse.mybir as mybir
import concourse.tile as tile
from concourse import bacc
from concourse.masks import make_identity

F32 = mybir.dt.float32
F32R = mybir.dt.float32r
BF16 = mybir.dt.bfloat16
I8 = mybir.dt.int8
AF = mybir.ActivationFunctionType
ALU = mybir.AluOpType

S = 128
E = 128
A = 8


def ts(i, size):
    return slice(i * size, (i + 1) * size)


def build_kernel(n_total=8192, m_core=1024):
    nc = bacc.Bacc("TRN2", debug=False)
    stateT = nc.dram_tensor("stateT", (S, n_total), BF16, kind="ExternalInput").ap()
    smbT = nc.dram_tensor("smbT", (S, m_core), BF16, kind="ExternalInput").ap()
    adjt = nc.dram_tensor("adjt", (n_total, m_core), I8, kind="ExternalInput").ap()
    wt = {}
    for name, shape in [
        ("w1", (S, E)), ("w2", (E, E)), ("wq", (E, E)), ("wk", (E, E)),
        ("wh1", (E, E)), ("wh2", (E, A)),
        ("b1", (E, 1)), ("b2", (E, 1)), ("bh1", (E, 1)), ("bh2", (A, 1)),
    ]:
        wt[name] = nc.dram_tensor(name, shape, F32, kind="ExternalInput").ap()
    outb = nc.dram_tensor("outb", (m_core, A), F32, kind="ExternalOutput").ap()

    with tile.TileContext(nc) as tc:
        colight_body(tc, outb, stateT, smbT, adjt, wt)
    nc.compile()
    return nc


def colight_body(tc, outb, stateT, smbT, adjt, wt):
    nc = tc.nc
    n_total = stateT.shape[1]
    m_core = adjt.shape[1]
    NT = n_total // 512   # n-groups of 512
    NB = n_total // 128   # n-blocks of 128
    MB = m_core // 128    # m-blocks of 128

    with ExitStack() as ctx:
        singles = ctx.enter_context(tc.tile_pool(name="singles", bufs=1))

        # ---- constants ----
        wf = {}
        for name, shape in [("w1", [S, E]), ("w2", [E, E]), ("wq", [E, E]),
                            ("wk", [E, E]), ("wh1", [E, E]), ("wh2", [E, A]),
                            ("b1", [E, 1]), ("b2", [E, 1]), ("bh1", [E, 1]),
                            ("bh2", [A, 1])]:
            t = singles.tile(shape, F32, tag=f"w_{name}")
            nc.scalar.dma_start(out=t, in_=wt[name])
            wf[name] = t
        w1b = singles.tile([S, E], BF16)
        w2b = singles.tile([E, E], BF16)
        wh1b = singles.tile([E, E], BF16)
        wh2b = singles.tile([E, A], BF16)
        wqr = singles.tile([E, E], F32R)
        wkr = singles.tile([E, E], F32R)
        for dst, src in [(w1b, "w1"), (w2b, "w2"), (wh1b, "wh1"), (wh2b, "wh2"),
                         (wqr, "wq"), (wkr, "wk")]:
            nc.vector.tensor_copy(out=dst, in_=wf[src])
        # bh1 as a 1-partition row (rank-1 bias update), bh2 broadcast tile
        bh1_row = singles.tile([1, E], F32)
        nc.scalar.dma_start(out=bh1_row, in_=wt["bh1"].rearrange("e o -> o e"))
        bh1_row_b = singles.tile([1, E], BF16)
        nc.vector.tensor_copy(out=bh1_row_b, in_=bh1_row)
        bh2_bcast = singles.tile([128, A], F32)
        nc.scalar.dma_start(
            out=bh2_bcast,
            in_=wt["bh2"].rearrange("a o -> o a").to_broadcast([128, A]),
        )
        ident = singles.tile([128, 128], F32)
        make_identity(nc, ident)
        identr = singles.tile([128, 128], F32R)
        nc.vector.tensor_copy(out=identr, in_=ident)
        ones_f = singles.tile([128, 1], F32)
        nc.vector.memset(ones_f, 1.0)
        ones_b = singles.tile([128, 1], BF16)
        nc.vector.tensor_copy(out=ones_b, in_=ones_f)

        # ---- persistent activations ----
        stT = singles.tile([S, n_total], BF16)
        nc.sync.dma_start(out=stT, in_=stateT)
        smbT_t = singles.tile([S, m_core], BF16)
        nc.sync.dma_start(out=smbT_t, in_=smbT)
        h1T = singles.tile([128, n_total], BF16)
        hT = singles.tile([128, n_total], F32R)
        kTs = singles.tile([E, n_total], F32R)
        qTs = singles.tile([E, m_core], F32R)
        hblk = singles.tile([128, NB, E], BF16)   # h in normal orientation
        agg_sb = singles.tile([128, m_core], BF16)
        den_row = singles.tile([1, m_core], F32)
        den_row_b = singles.tile([1, m_core], BF16)

        ph1_stack = ExitStack()
        ph1_sb = ph1_stack.enter_context(tc.tile_pool(name="ph1_sb", bufs=3))
        ph1_ps = ph1_stack.enter_context(tc.tile_pool(name="ph1_ps", bufs=4, space="PSUM"))

        # ---- q-path first (phase 2 needs qTs for every block) ----
        for j in range(m_core // 512):
            ps1 = ph1_ps.tile([128, 512], F32, tag="p")
            nc.tensor.matmul(ps1, w1b, smbT_t[:, ts(j, 512)], start=True, stop=True)
            h1m = ph1_sb.tile([128, 512], BF16, tag="sbq")
            nc.scalar.activation(h1m, ps1, AF.Relu, bias=wf["b1"], scale=1.0)
            ps2 = ph1_ps.tile([128, 512], F32, tag="p")
            nc.tensor.matmul(ps2, w2b, h1m, start=True, stop=True)
            hm = ph1_sb.tile([128, 512], F32R, tag="sbq")
            nc.scalar.activation(hm, ps2, AF.Identity, bias=wf["b2"], scale=1.0)
            ps3 = ph1_ps.tile([128, 512], F32, tag="p")
            nc.tensor.matmul(ps3, wqr, hm, start=True, stop=True)
            nc.scalar.copy(out=qTs[:, ts(j, 512)], in_=ps3)

        # ---- fused per-512 pipeline: h1T -> hT -> kT -> h transposes ----
        for j in range(NT):
            ps = ph1_ps.tile([128, 512], F32, tag="p")
            nc.tensor.matmul(ps, w1b, stT[:, ts(j, 512)], start=True, stop=True)
            nc.scalar.activation(h1T[:, ts(j, 512)], ps, AF.Relu, bias=wf["b1"], scale=1.0)
            ps2 = ph1_ps.tile([128, 512], F32, tag="p")
            nc.tensor.matmul(ps2, w2b, h1T[:, ts(j, 512)], start=True, stop=True)
            nc.scalar.activation(hT[:, ts(j, 512)], ps2, AF.Identity, bias=wf["b2"], scale=1.0)
            ps3 = ph1_ps.tile([128, 512], F32, tag="p")
            nc.tensor.matmul(ps3, wkr, hT[:, ts(j, 512)], start=True, stop=True)
            nc.vector.tensor_copy(out=kTs[:, ts(j, 512)], in_=ps3)
            hx = ph1_ps.tile([128, 512], F32R, tag="p")
            for a in range(4):
                nb = j * 4 + a
                nc.tensor.transpose(hx[:, ts(a, 128)], hT[:, ts(nb, 128)], identr)
            nc.vector.tensor_copy(
                out=hblk[:, ts(j, 4), :],
                in_=hx.rearrange("p (a e) -> p a e", a=4),
            )

        ph1_stack.close()

        # ---- phase 2: transposed masked attention over both m-halves ----
        ph2_stack = ExitStack()
        adj_pool = ph2_stack.enter_context(tc.tile_pool(name="adj", bufs=3))
        exp_pool = ph2_stack.enter_context(tc.tile_pool(name="expT", bufs=8))
        sc_ps = ph2_stack.enter_context(tc.tile_pool(name="sc_ps", bufs=2, space="PSUM"))
        agg_psp = ph2_stack.enter_context(tc.tile_pool(name="agg_ps", bufs=1, space="PSUM"))

        agg0 = agg_psp.tile([128, 512], F32, tag="agg0")
        agg1 = agg_psp.tile([128, 512], F32, tag="agg1")
        den0 = agg_psp.tile([1, 512], F32, tag="den0")
        den1 = agg_psp.tile([1, 512], F32, tag="den1")

        for ng in range(NT):
            adjt_t = adj_pool.tile([128, 4, m_core], I8)
            nc.sync.dma_start(
                out=adjt_t,
                in_=adjt[ts(ng, 512), :].rearrange("(a p) m -> p a m", p=128),
            )
            for a in range(4):
                nb = ng * 4 + a
                scp = sc_ps.tile([128, m_core], F32)
                nc.tensor.matmul(
                    scp[:, 0:512], kTs[:, ts(nb, 128)], qTs[:, 0:512],
                    start=True, stop=True,
                )
                nc.tensor.matmul(
                    scp[:, 512:1024], kTs[:, ts(nb, 128)], qTs[:, 512:1024],
                    start=True, stop=True,
                )
                exps = exp_pool.tile([128, m_core], BF16, tag="exps")
                nc.scalar.activation(exps, scp, AF.Exp, bias=0.0, scale=1.0)
                expT = exp_pool.tile([128, m_core], BF16, tag="expm")
                nc.vector.tensor_tensor(
                    out=expT, in0=exps, in1=adjt_t[:, a, :], op=ALU.mult,
                )
                nc.tensor.matmul(
                    agg0, hblk[:, nb, :], expT[:, 0:512],
                    start=(nb == 0), stop=(nb == NB - 1),
                )
                nc.tensor.matmul(
                    agg1, hblk[:, nb, :], expT[:, 512:1024],
                    start=(nb == 0), stop=(nb == NB - 1),
                )
                nc.tensor.matmul(
                    den0, ones_b, expT[:, 0:512],
                    start=(nb == 0), stop=(nb == NB - 1),
                )
                nc.tensor.matmul(
                    den1, ones_b, expT[:, 512:1024],
                    start=(nb == 0), stop=(nb == NB - 1),
                )

        # aggT / den out of PSUM
        nc.vector.tensor_copy(out=agg_sb[:, 0:512], in_=agg0)
        nc.vector.tensor_copy(out=agg_sb[:, 512:1024], in_=agg1)
        for half, dps in ((0, den0), (1, den1)):
            nc.vector.tensor_copy(out=den_row[0:1, ts(half, 512)], in_=dps)
            nc.vector.tensor_copy(out=den_row_b[0:1, ts(half, 512)], in_=dps)
        ph2_stack.close()

        # ---- head ----
        head_sb = ctx.enter_context(tc.tile_pool(name="head_sb", bufs=3))
        head_ps = ctx.enter_context(tc.tile_pool(name="head_ps", bufs=2, space="PSUM"))
        head_ps2 = ctx.enter_context(tc.tile_pool(name="head_ps2", bufs=1, space="PSUM"))

        for mb in range(MB):
            # h3T_unnorm = wh1^T@aggT + bh1 (x) den   (PSUM-accumulated)
            h3_ps = head_ps.tile([128, 128], F32, tag="hps")
            nc.tensor.matmul(h3_ps, wh1b, agg_sb[:, ts(mb, 128)],
                             start=True, stop=False)
            nc.tensor.matmul(h3_ps, bh1_row_b, den_row_b[0:1, ts(mb, 128)],
                             start=False, stop=True)
            h3 = head_sb.tile([128, 128], BF16, tag="hsb")
            nc.scalar.activation(h3, h3_ps, AF.Relu, bias=0.0, scale=1.0)
            oT_ps = head_ps.tile([8, 128], F32, tag="ops")
            nc.tensor.matmul(oT_ps, wh2b, h3, start=True, stop=True)
            oT = head_sb.tile([8, 128], F32, tag="osb")
            nc.vector.tensor_copy(out=oT, in_=oT_ps)
            # per-m reciprocal of den
            denT_ps = head_ps2.tile([128, 1], F32, tag="dtp")
            nc.tensor.transpose(denT_ps, den_row[0:1, ts(mb, 128)], ident[0:1, 0:1])
            rden = head_sb.tile([128, 1], F32, tag="rdn")
            nc.vector.reciprocal(rden, denT_ps)
            o_ps = head_ps2.tile([128, A], F32, tag="otp")
            nc.tensor.transpose(o_ps, oT, ident[0:8, 0:8])
            o_sb = head_sb.tile([128, A], F32, tag="fin")
            nc.vector.scalar_tensor_tensor(
                out=o_sb, in0=o_ps, scalar=rden, in1=bh2_bcast,
                op0=ALU.mult, op1=ALU.add,
            )
            nc.scalar.dma_start(out=outb[ts(mb, 128), :], in_=o_sb)


# ----------------------------------------------------------------------------
# Host entry point: full inputs in, full output out. 8-way row sharding.
# ----------------------------------------------------------------------------
import numpy as np
import ml_dtypes

N_TOTAL = 8192
N_CORES = 8
M_CORE = N_TOTAL // N_CORES

_cached = {}


def _get_nc():
    if "nc" not in _cached:
        _cached["nc"] = build_kernel(n_total=N_TOTAL, m_core=M_CORE)
    return _cached["nc"]


def make_in_maps(state_matrix, adj, w1, b1, w2, b2, wq, wk, wh1, bh1, wh2, bh2):
    state_matrix = np.asarray(state_matrix, dtype=np.float32)
    stateT = np.ascontiguousarray(state_matrix.T).astype(ml_dtypes.bfloat16)
    adj = np.asarray(adj)
    f32 = lambda x: np.ascontiguousarray(np.asarray(x, dtype=np.float32))
    wq_scaled = f32(wq) / np.float32(np.sqrt(E))
    # transposed adjacency as int8: adjt_full[n, m] = adj[m, n]
    adjt_full = np.ascontiguousarray(adj.T.astype(np.int8))
    common = {
        "stateT": stateT,
        "w1": f32(w1), "w2": f32(w2), "wq": wq_scaled, "wk": f32(wk),
        "wh1": f32(wh1), "wh2": f32(wh2),
        "b1": f32(b1).reshape(E, 1), "b2": f32(b2).reshape(E, 1),
        "bh1": f32(bh1).reshape(E, 1), "bh2": f32(bh2).reshape(A, 1),
    }
    in_maps = []
    for c in range(N_CORES):
        rows = slice(c * M_CORE, (c + 1) * M_CORE)
        in_maps.append(
            dict(
                common,
                smbT=np.ascontiguousarray(stateT[:, rows]),
                adjt=np.ascontiguousarray(adjt_full[:, rows]),
            )
        )
    return in_maps


def kernel(state_matrix, adj, w1, b1, w2, b2, wq, wk, wh1, bh1, wh2, bh2):
    from concourse import bass_utils

    in_maps = make_in_maps(
        state_matrix, adj, w1, b1, w2, b2, wq, wk, wh1, bh1, wh2, bh2
    )
    res = bass_utils.run_bass_kernel_spmd(
        _get_nc(), in_maps, core_ids=list(range(N_CORES))
    )
    out = np.concatenate([r["outb"] for r in res.results], axis=0)
    return out.astype(np.float32)


# revision 9
# speedup vs baseline: 1.6146x; 1.1328x over previous
"""CoLightNet Trainium2 Bass kernel (self-contained).

SPMD over 8 cores; core c owns output rows [c*1024, (c+1)*1024).
  inputs : stateT   [S,N]    bf16 (replicated, host-transposed state)
           smbT     [S,Mc]   bf16 (core's own row block, transposed)
           adjt     [N,Mc]   i8   (transposed adjacency: adj[rows].T)
           w1,w2,wq,wk,wh1 [128,128], wh2 [128,8], biases as [*,1] f32
           (wq is pre-scaled by 1/sqrt(E) on the host)
  output : outb     [Mc,A]   f32

Math (identical to the reference, reformulated):
  h    = relu(state@w1+b1)@w2+b2
  sT   = (h@wk)^T-stationary x (h@wq/sqrt(E))-moving       # scores^T [n,m]
  wT   = exp(sT) * adj^T                                   # exact masking
  aggT = h^T-stationary x wT-moving  (per n-block, PSUM-accumulated)
  den  = column-sum of wT (GpSimd-accumulated, PE-reduced)
  head : relu(agg/den @ wh1 + bh1) @ wh2 + bh2
       = [relu(agg@wh1 + bh1 (x) den) / den] @ wh2 + bh2   # rank-1 bias trick

MLP/attention-weight matmuls run bf16 (moving dim 512); the q/k
projections stay float32r so the pre-exp scores keep fp32-level accuracy.
"""

from contextlib import ExitStack

import concourse.bass as bass
import concourse.mybir as mybir
import concourse.tile as tile
from concourse import bacc
from concourse.masks import make_identity

F32 = mybir.dt.float32
F32R = mybir.dt.float32r
BF16 = mybir.dt.bfloat16
I8 = mybir.dt.int8
AF = mybir.ActivationFunctionType
ALU = mybir.AluOpType

S = 128
E = 128
A = 8


def ts(i, size):
    return slice(i * size, (i + 1) * size)


def build_kernel(n_total=8192, m_core=1024):
    nc = bacc.Bacc("TRN2", debug=False)
    stateT = nc.dram_tensor("stateT", (S, n_total), BF16, kind="ExternalInput").ap()
    smbT = nc.dram_tensor("smbT", (S, m_core), BF16, kind="ExternalInput").ap()
    adjt = nc.dram_tensor("adjt", (n_total, m_core), I8, kind="ExternalInput").ap()
    wt = {}
    for name, shape in [
        ("w1", (S, E)), ("w2", (E, E)), ("wq", (E, E)), ("wk", (E, E)),
        ("wh1", (E, E)), ("wh2", (E, A)),
        ("b1", (E, 1)), ("b2", (E, 1)), ("bh1", (E, 1)), ("bh2", (A, 1)),
    ]:
        wt[name] = nc.dram_tensor(name, shape, F32, kind="ExternalInput").ap()
    outb = nc.dram_tensor("outb", (m_core, A), F32, kind="ExternalOutput").ap()

    with tile.TileContext(nc) as tc:
        colight_body(tc, outb, stateT, smbT, adjt, wt)
    nc.compile()
    return nc


def colight_body(tc, outb, stateT, smbT, adjt, wt):
    nc = tc.nc
    n_total = stateT.shape[1]
    m_core = adjt.shape[1]
    NT = n_total // 512   # n-groups of 512
    NB = n_total // 128   # n-blocks of 128
    MB = m_core // 128    # m-blocks of 128

    with ExitStack() as ctx:
        singles = ctx.enter_context(tc.tile_pool(name="singles", bufs=1))

        # ---- constants ----
        wf = {}
        for name, shape in [("w1", [S, E]), ("w2", [E, E]), ("wq", [E, E]),
                            ("wk", [E, E]), ("wh1", [E, E]), ("wh2", [E, A]),
                            ("b1", [E, 1]), ("b2", [E, 1]), ("bh1", [E, 1]),
                            ("bh2", [A, 1])]:
            t = singles.tile(shape, F32, tag=f"w_{name}")
            nc.scalar.dma_start(out=t, in_=wt[name])
            wf[name] = t
        w1b = singles.tile([S, E], BF16)
        w2b = singles.tile([E, E], BF16)
        wh1b = singles.tile([E, E], BF16)
        wh2b = singles.tile([E, A], BF16)
        wqr = singles.tile([E, E], F32R)
        wkr = singles.tile([E, E], F32R)
        for dst, src in [(w1b, "w1"), (w2b, "w2"), (wh1b, "wh1"), (wh2b, "wh2"),
                         (wqr, "wq"), (wkr, "wk")]:
            nc.vector.tensor_copy(out=dst, in_=wf[src])
        # bh1 as a 1-partition row (rank-1 bias update), bh2 broadcast tile
        bh1_row = singles.tile([1, E], F32)
        nc.scalar.dma_start(out=bh1_row, in_=wt["bh1"].rearrange("e o -> o e"))
        bh1_row_b = singles.tile([1, E], BF16)
        nc.vector.tensor_copy(out=bh1_row_b, in_=bh1_row)
        bh2_bcast = singles.tile([128, A], F32)
        nc.scalar.dma_start(
            out=bh2_bcast,
            in_=wt["bh2"].rearrange("a o -> o a").to_broadcast([128, A]),
        )
        ident = singles.tile([128, 128], F32)
        make_identity(nc, ident)
        identr = singles.tile([128, 128], F32R)
        nc.vector.tensor_copy(out=identr, in_=ident)
        ones_f = singles.tile([128, 1], F32)
        nc.vector.memset(ones_f, 1.0)
        ones_b = singles.tile([128, 1], BF16)
        nc.vector.tensor_copy(out=ones_b, in_=ones_f)

        # ---- persistent activations ----
        stT = singles.tile([S, n_total], BF16)
        nc.sync.dma_start(out=stT, in_=stateT)
        smbT_t = singles.tile([S, m_core], BF16)
        nc.sync.dma_start(out=smbT_t, in_=smbT)
        h1T = singles.tile([128, n_total], BF16)
        hT = singles.tile([128, n_total], F32R)
        kTs = singles.tile([E, n_total], F32R)
        qTs = singles.tile([E, m_core], F32R)
        hblk = singles.tile([128, NB, E], BF16)   # h in normal orientation
        agg_sb = singles.tile([128, m_core], BF16)
        den_row = singles.tile([1, m_core], F32)
        den_row_b = singles.tile([1, m_core], BF16)

        ph2_stack = ExitStack()
        ph1_sb = ph2_stack.enter_context(tc.tile_pool(name="ph1_sb", bufs=3))
        ph1_ps = ph2_stack.enter_context(tc.tile_pool(name="ph1_ps", bufs=2, space="PSUM"))
        adj_pool = ph2_stack.enter_context(tc.tile_pool(name="adj", bufs=4))
        exp_pool = ph2_stack.enter_context(tc.tile_pool(name="expT", bufs=8))
        sc_ps = ph2_stack.enter_context(tc.tile_pool(name="sc_ps", bufs=3, space="PSUM"))
        agg_psp = ph2_stack.enter_context(tc.tile_pool(name="agg_ps", bufs=1, space="PSUM"))

        agg0 = agg_psp.tile([128, 512], F32, tag="agg0")
        agg1 = agg_psp.tile([128, 512], F32, tag="agg1")
        den_ps = agg_psp.tile([64, 512], F32, tag="den")
        den0 = den_ps[0:1, :]
        den1 = den_ps[32:33, :]

        # prefetch the first adjacency tiles while phase 1 runs
        adj_tiles = {}
        def fetch_adj(ng):
            t = adj_pool.tile([128, 4, m_core], I8)
            nc.sync.dma_start(
                out=t,
                in_=adjt[ts(ng, 512), :].rearrange("(a p) m -> p a m", p=128),
            )
            adj_tiles[ng] = t

        fetch_adj(0)
        fetch_adj(1)

        # ---- q-path first (phase 2 needs qTs for every block) ----
        for j in range(m_core // 512):
            ps1 = ph1_ps.tile([128, 512], F32, tag="p")
            nc.tensor.matmul(ps1, w1b, smbT_t[:, ts(j, 512)], start=True, stop=True)
            h1m = ph1_sb.tile([128, 512], BF16, tag="sbq")
            nc.scalar.activation(h1m, ps1, AF.Relu, bias=wf["b1"], scale=1.0)
            ps2 = ph1_ps.tile([128, 512], F32, tag="p")
            nc.tensor.matmul(ps2, w2b, h1m, start=True, stop=True)
            hm = ph1_sb.tile([128, 512], F32R, tag="sbq")
            nc.scalar.activation(hm, ps2, AF.Identity, bias=wf["b2"], scale=1.0)
            ps3 = ph1_ps.tile([128, 512], F32, tag="p")
            nc.tensor.matmul(ps3, wqr, hm, start=True, stop=True)
            nc.scalar.copy(out=qTs[:, ts(j, 512)], in_=ps3)

        # ---- fused pipeline: phase-1 group j, then attention n-group j ----
        for j in range(NT):
            # phase 1 for columns [512j, 512j+512): h1T -> hT -> kT -> hblk
            ps = ph1_ps.tile([128, 512], F32, tag="p")
            nc.tensor.matmul(ps, w1b, stT[:, ts(j, 512)], start=True, stop=True)
            nc.scalar.activation(h1T[:, ts(j, 512)], ps, AF.Relu, bias=wf["b1"], scale=1.0)
            ps2 = ph1_ps.tile([128, 512], F32, tag="p")
            nc.tensor.matmul(ps2, w2b, h1T[:, ts(j, 512)], start=True, stop=True)
            nc.scalar.activation(hT[:, ts(j, 512)], ps2, AF.Identity, bias=wf["b2"], scale=1.0)
            ps3 = ph1_ps.tile([128, 512], F32, tag="p")
            nc.tensor.matmul(ps3, wkr, hT[:, ts(j, 512)], start=True, stop=True)
            nc.vector.tensor_copy(out=kTs[:, ts(j, 512)], in_=ps3)
            hx = ph1_ps.tile([128, 512], F32R, tag="p")
            for a in range(4):
                nb = j * 4 + a
                nc.tensor.transpose(hx[:, ts(a, 128)], hT[:, ts(nb, 128)], identr)
            nc.vector.tensor_copy(
                out=hblk[:, ts(j, 4), :],
                in_=hx.rearrange("p (a e) -> p a e", a=4),
            )

            # phase 2 for n-group j (kTs/hblk for these blocks just landed)
            if j + 2 < NT:
                fetch_adj(j + 2)
            adjt_t = adj_tiles.pop(j)
            for a in range(4):
                nb = j * 4 + a
                scpA = sc_ps.tile([128, 512], F32, tag="sc")
                nc.tensor.matmul(
                    scpA, kTs[:, ts(nb, 128)], qTs[:, 0:512],
                    start=True, stop=True,
                )
                scpB = sc_ps.tile([128, 512], F32, tag="sc")
                nc.tensor.matmul(
                    scpB, kTs[:, ts(nb, 128)], qTs[:, 512:1024],
                    start=True, stop=True,
                )
                exps = exp_pool.tile([128, m_core], BF16, tag="exps")
                nc.scalar.activation(exps[:, 0:512], scpA, AF.Exp, bias=0.0, scale=1.0)
                nc.scalar.activation(exps[:, 512:1024], scpB, AF.Exp, bias=0.0, scale=1.0)
                expT = exp_pool.tile([128, m_core], BF16, tag="expm")
                nc.vector.tensor_tensor(
                    out=expT, in0=exps, in1=adjt_t[:, a, :], op=ALU.mult,
                )
                nc.tensor.matmul(
                    agg0, hblk[:, nb, :], expT[:, 0:512],
                    start=(nb == 0), stop=(nb == NB - 1),
                )
                nc.tensor.matmul(
                    agg1, hblk[:, nb, :], expT[:, 512:1024],
                    start=(nb == 0), stop=(nb == NB - 1),
                )
                nc.tensor.matmul(
                    den0, ones_b, expT[:, 0:512],
                    start=(nb == 0), stop=(nb == NB - 1),
                )
                nc.tensor.matmul(
                    den1, ones_b, expT[:, 512:1024],
                    start=(nb == 0), stop=(nb == NB - 1),
                )

        # aggT / den out of PSUM
        nc.vector.tensor_copy(out=agg_sb[:, 0:512], in_=agg0)
        nc.vector.tensor_copy(out=agg_sb[:, 512:1024], in_=agg1)
        for half, dps in ((0, den0), (1, den1)):
            nc.vector.tensor_copy(out=den_row[0:1, ts(half, 512)], in_=dps)
            nc.vector.tensor_copy(out=den_row_b[0:1, ts(half, 512)], in_=dps)
        ph2_stack.close()

        # ---- head ----
        head_sb = ctx.enter_context(tc.tile_pool(name="head_sb", bufs=3))
        head_ps = ctx.enter_context(tc.tile_pool(name="head_ps", bufs=2, space="PSUM"))
        head_ps2 = ctx.enter_context(tc.tile_pool(name="head_ps2", bufs=1, space="PSUM"))

        for mb in range(MB):
            # h3T_unnorm = wh1^T@aggT + bh1 (x) den   (PSUM-accumulated)
            h3_ps = head_ps.tile([128, 128], F32, tag="hps")
            nc.tensor.matmul(h3_ps, wh1b, agg_sb[:, ts(mb, 128)],
                             start=True, stop=False)
            nc.tensor.matmul(h3_ps, bh1_row_b, den_row_b[0:1, ts(mb, 128)],
                             start=False, stop=True)
            h3 = head_sb.tile([128, 128], BF16, tag="hsb")
            nc.scalar.activation(h3, h3_ps, AF.Relu, bias=0.0, scale=1.0)
            oT_ps = head_ps.tile([8, 128], F32, tag="ops")
            nc.tensor.matmul(oT_ps, wh2b, h3, start=True, stop=True)
            oT = head_sb.tile([8, 128], F32, tag="osb")
            nc.vector.tensor_copy(out=oT, in_=oT_ps)
            # per-m reciprocal of den
            denT_ps = head_ps2.tile([128, 1], F32, tag="dtp")
            nc.tensor.transpose(denT_ps, den_row[0:1, ts(mb, 128)], ident[0:1, 0:1])
            rden = head_sb.tile([128, 1], F32, tag="rdn")
            nc.vector.reciprocal(rden, denT_ps)
            o_ps = head_ps2.tile([128, A], F32, tag="otp")
            nc.tensor.transpose(o_ps, oT, ident[0:8, 0:8])
            o_sb = head_sb.tile([128, A], F32, tag="fin")
            nc.vector.scalar_tensor_tensor(
                out=o_sb, in0=o_ps, scalar=rden, in1=bh2_bcast,
                op0=ALU.mult, op1=ALU.add,
            )
            nc.scalar.dma_start(out=outb[ts(mb, 128), :], in_=o_sb)


# ----------------------------------------------------------------------------
# Host entry point: full inputs in, full output out. 8-way row sharding.
# ----------------------------------------------------------------------------
import numpy as np
import ml_dtypes

N_TOTAL = 8192
N_CORES = 8
M_CORE = N_TOTAL // N_CORES

_cached = {}


def _get_nc():
    if "nc" not in _cached:
        _cached["nc"] = build_kernel(n_total=N_TOTAL, m_core=M_CORE)
    return _cached["nc"]


def make_in_maps(state_matrix, adj, w1, b1, w2, b2, wq, wk, wh1, bh1, wh2, bh2):
    state_matrix = np.asarray(state_matrix, dtype=np.float32)
    stateT = np.ascontiguousarray(state_matrix.T).astype(ml_dtypes.bfloat16)
    adj = np.asarray(adj)
    f32 = lambda x: np.ascontiguousarray(np.asarray(x, dtype=np.float32))
    wq_scaled = f32(wq) / np.float32(np.sqrt(E))
    # transposed adjacency as int8: adjt_full[n, m] = adj[m, n]
    adjt_full = np.ascontiguousarray(adj.T.astype(np.int8))
    common = {
        "stateT": stateT,
        "w1": f32(w1), "w2": f32(w2), "wq": wq_scaled, "wk": f32(wk),
        "wh1": f32(wh1), "wh2": f32(wh2),
        "b1": f32(b1).reshape(E, 1), "b2": f32(b2).reshape(E, 1),
        "bh1": f32(bh1).reshape(E, 1), "bh2": f32(bh2).reshape(A, 1),
    }
    in_maps = []
    for c in range(N_CORES):
        rows = slice(c * M_CORE, (c + 1) * M_CORE)
        in_maps.append(
            dict(
                common,
                smbT=np.ascontiguousarray(stateT[:, rows]),
                adjt=np.ascontiguousarray(adjt_full[:, rows]),
            )
        )
    return in_maps


def kernel(state_matrix, adj, w1, b1, w2, b2, wq, wk, wh1, bh1, wh2, bh2):
    from concourse import bass_utils

    in_maps = make_in_maps(
        state_matrix, adj, w1, b1, w2, b2, wq, wk, wh1, bh1, wh2, bh2
    )
    res = bass_utils.run_bass_kernel_spmd(
        _get_nc(), in_maps, core_ids=list(range(N_CORES))
    )
    out = np.concatenate([r["outb"] for r in res.results], axis=0)
    return out.astype(np.float32)


# revision 13
# speedup vs baseline: 2.0693x; 1.2816x over previous
"""CoLightNet Trainium2 Bass kernel (self-contained).

SPMD over 8 cores; core c owns output rows [c*1024, (c+1)*1024).
  inputs : stateT   [S,N]    bf16 (replicated, host-transposed state)
           smbT     [S,Mc]   bf16 (core's own row block, transposed)
           adjm     [N,Mc]   i16  (Schraudolph mask: 56 if edge else -32768)
           w1,w2,wq,wk,wh1 [128,128], wh2 [128,8], biases as [*,1] f32
           (wq is pre-scaled by A8/sqrt(E), A8 = 8/ln2)
  output : outb     [Mc,A]   f32

Math (reference, reformulated):
  h    = relu(state@w1+b1)@w2+b2
  sT   = (h@wk)^T-stationary x (h@wq*A8/sqrt(E))-moving    # A8-scaled scores^T
  wT   = bitcast_fp8e4(int8(round(sT + adjm)))             # Schraudolph exp:
         # edge:   int8(s*A8+56)  bitcast e4m3 ~= exp(s) (+-4%, ratio-cancels)
         # masked: saturates to -128 = -0.0 in fp8         -> exact zero weight
  aggT = h^T(fp8) x wT  via fp8 DoubleRow (2 n-blocks per matmul, 2 rows/cyc)
  den  = ones(fp8) x wT via fp8 DoubleRow
  head : relu(agg/den @ wh1 + bh1) @ wh2 + bh2
       = [relu(agg@wh1 + bh1 (x) den) / den] @ wh2 + bh2   # rank-1 bias trick

Engine budget per 128-node block: Vector 1x STT (mask+exp fused), PE
score 2x216ns + DR agg/den ~4x213ns per block pair, Scalar only phase-1/
head activations + PSUM->SBUF copies. Phase 1 and phase 2 are interleaved
per 512-column group so no engine sits idle behind the MLP chain.
"""

from contextlib import ExitStack

import concourse.bass as bass
import concourse.mybir as mybir
import concourse.tile as tile
from concourse import bacc
from concourse.masks import make_identity

F32 = mybir.dt.float32
F32R = mybir.dt.float32r
BF16 = mybir.dt.bfloat16
FP8 = mybir.dt.float8e4
I8 = mybir.dt.int8
I16 = mybir.dt.int16
AF = mybir.ActivationFunctionType
ALU = mybir.AluOpType
DR = mybir.MatmulPerfMode.DoubleRow

S = 128
E = 128
A = 8


def ts(i, size):
    return slice(i * size, (i + 1) * size)


def build_kernel(n_total=8192, m_core=1024):
    nc = bacc.Bacc("TRN2", debug=False)
    stateT = nc.dram_tensor("stateT", (S, n_total), BF16, kind="ExternalInput").ap()
    smbT = nc.dram_tensor("smbT", (S, m_core), BF16, kind="ExternalInput").ap()
    adjm = nc.dram_tensor("adjm", (n_total, m_core), I16, kind="ExternalInput").ap()
    wt = {}
    for name, shape in [
        ("w1", (S, E)), ("w2", (E, E)), ("wq", (E, E)), ("wk", (E, E)),
        ("wh1", (E, E)), ("wh2", (E, A)),
        ("b1", (E, 1)), ("b2", (E, 1)), ("bh1", (E, 1)), ("bh2", (A, 1)),
    ]:
        wt[name] = nc.dram_tensor(name, shape, F32, kind="ExternalInput").ap()
    outb = nc.dram_tensor("outb", (m_core, A), F32, kind="ExternalOutput").ap()

    with tile.TileContext(nc) as tc:
        colight_body(tc, outb, stateT, smbT, adjm, wt)
    nc.compile()
    return nc


def colight_body(tc, outb, stateT, smbT, adjm, wt):
    nc = tc.nc
    n_total = stateT.shape[1]
    m_core = adjm.shape[1]
    NT = n_total // 512   # n-groups of 512
    NB = n_total // 128   # n-blocks of 128
    NU = NB // 2          # n-block pairs (DoubleRow granule)
    MB = m_core // 128    # m-blocks of 128

    with ExitStack() as ctx:
        singles = ctx.enter_context(tc.tile_pool(name="singles", bufs=1))

        # ---- constants ----
        wf = {}
        for name, shape in [("w1", [S, E]), ("w2", [E, E]), ("wq", [E, E]),
                            ("wk", [E, E]), ("wh1", [E, E]), ("wh2", [E, A]),
                            ("b1", [E, 1]), ("b2", [E, 1]), ("bh1", [E, 1]),
                            ("bh2", [A, 1])]:
            t = singles.tile(shape, F32, tag=f"w_{name}")
            nc.scalar.dma_start(out=t, in_=wt[name])
            wf[name] = t
        w1b = singles.tile([S, E], BF16)
        w2b = singles.tile([E, E], BF16)
        wh1b = singles.tile([E, E], BF16)
        wh2b = singles.tile([E, A], BF16)
        wqr = singles.tile([E, E], F32R)
        wkr = singles.tile([E, E], F32R)
        for dst, src in [(w1b, "w1"), (w2b, "w2"), (wh1b, "wh1"), (wh2b, "wh2"),
                         (wqr, "wq"), (wkr, "wk")]:
            nc.vector.tensor_copy(out=dst, in_=wf[src])
        # bh1 as a 1-partition row (rank-1 bias update), bh2 broadcast tile
        bh1_row = singles.tile([1, E], F32)
        nc.scalar.dma_start(out=bh1_row, in_=wt["bh1"].rearrange("e o -> o e"))
        bh1_row_b = singles.tile([1, E], BF16)
        nc.vector.tensor_copy(out=bh1_row_b, in_=bh1_row)
        bh2_bcast = singles.tile([128, A], F32)
        nc.scalar.dma_start(
            out=bh2_bcast,
            in_=wt["bh2"].rearrange("a o -> o a").to_broadcast([128, A]),
        )
        ident = singles.tile([128, 128], F32)
        make_identity(nc, ident)
        ones8 = singles.tile([128, 1], FP8)
        nc.vector.memset(ones8, 1.0)

        # ---- persistent activations ----
        stT = singles.tile([S, n_total], BF16)
        nc.sync.dma_start(out=stT, in_=stateT)
        smbT_t = singles.tile([S, m_core], BF16)
        nc.sync.dma_start(out=smbT_t, in_=smbT)
        h1T = singles.tile([128, n_total], BF16)
        hT = singles.tile([128, n_total], F32R)
        kTs = singles.tile([E, n_total], F32R)
        qTs = singles.tile([E, m_core], F32R)
        hblk8 = singles.tile([128, NU, 2, E], FP8)  # h, DR-paired [u][i][e]
        agg_sb = singles.tile([128, m_core], BF16)
        den_row = singles.tile([1, m_core], F32)
        den_row_b = singles.tile([1, m_core], BF16)

        ph2_stack = ExitStack()
        ph1_sb = ph2_stack.enter_context(tc.tile_pool(name="ph1_sb", bufs=3))
        ph1_ps = ph2_stack.enter_context(tc.tile_pool(name="ph1_ps", bufs=1, space="PSUM"))
        adj_pool = ph2_stack.enter_context(tc.tile_pool(name="adjm", bufs=4))
        exp_pool = ph2_stack.enter_context(tc.tile_pool(name="expp", bufs=3))
        sc_ps = ph2_stack.enter_context(tc.tile_pool(name="sc_ps", bufs=2, space="PSUM"))
        agg_psp = ph2_stack.enter_context(tc.tile_pool(name="agg_ps", bufs=1, space="PSUM"))

        agg0 = agg_psp.tile([128, 512], F32, tag="agg0")
        agg1 = agg_psp.tile([128, 512], F32, tag="agg1")
        den_ps = agg_psp.tile([64, 512], F32, tag="den")
        den0 = den_ps[0:1, :]
        den1 = den_ps[32:33, :]

        adj_tiles = {}
        def fetch_adjm(ng):
            t = adj_pool.tile([128, 4, m_core], I16)
            for a in range(4):
                nc.sync.dma_start(
                    out=t[:, a, :],
                    in_=adjm[512 * ng + 128 * a: 512 * ng + 128 * (a + 1), :],
                )
            adj_tiles[ng] = t

        fetch_adjm(0)
        fetch_adjm(1)

        exp_tiles = {}
        def ph2_block(ng, a):
            nb = ng * 4 + a
            u, i = nb // 2, nb % 2
            adjt_t = adj_tiles[ng]
            scp = sc_ps.tile([128, m_core], F32, tag="sc")
            nc.tensor.matmul(scp[:, 0:512], kTs[:, ts(nb, 128)], qTs[:, 0:512],
                             start=True, stop=True)
            nc.tensor.matmul(scp[:, 512:1024], kTs[:, ts(nb, 128)], qTs[:, 512:1024],
                             start=True, stop=True)
            if i == 0:
                exp_tiles[u] = exp_pool.tile([128, 2, m_core], I8, tag="e", name=f"ep{u}")
            ep = exp_tiles[u]
            # fused mask + Schraudolph exp: int8(round(s*A8 + {56|-32768}))
            nc.vector.tensor_tensor(out=ep[:, i, :], in0=scp, in1=adjt_t[:, a, :],
                                    op=ALU.add)
            nc.tensor.matmul(den0, ones8, ep[:, i, 0:512].bitcast(FP8),
                             start=(nb == 0), stop=(nb == NB - 1))
            nc.tensor.matmul(den1, ones8, ep[:, i, 512:1024].bitcast(FP8),
                             start=(nb == 0), stop=(nb == NB - 1))
            if i == 1:
                mov0 = ep[:, :, 0:512].bitcast(FP8)
                mov1 = ep[:, :, 512:1024].bitcast(FP8)
                st8 = hblk8[:, u, :, :]
                nc.tensor.matmul(agg0, st8, mov0, start=(u == 0), stop=(u == NU - 1),
                                 perf_mode=DR)
                nc.tensor.matmul(agg1, st8, mov1, start=(u == 0), stop=(u == NU - 1),
                                 perf_mode=DR)
                del exp_tiles[u]
                if a == 3:
                    del adj_tiles[ng]

        # ---- q-path first (phase 2 needs qTs for every block) ----
        for j in range(m_core // 512):
            ps1 = ph1_ps.tile([128, 512], F32, tag="p")
            nc.tensor.matmul(ps1, w1b, smbT_t[:, ts(j, 512)], start=True, stop=True)
            h1m = ph1_sb.tile([128, 512], BF16, tag="sbq")
            nc.scalar.activation(h1m, ps1, AF.Relu, bias=wf["b1"], scale=1.0)
            ps2 = ph1_ps.tile([128, 512], F32, tag="p")
            nc.tensor.matmul(ps2, w2b, h1m, start=True, stop=True)
            hm = ph1_sb.tile([128, 512], F32R, tag="sbq")
            nc.scalar.activation(hm, ps2, AF.Identity, bias=wf["b2"], scale=1.0)
            ps3 = ph1_ps.tile([128, 512], F32, tag="p")
            nc.tensor.matmul(ps3, wqr, hm, start=True, stop=True)
            nc.scalar.copy(out=qTs[:, ts(j, 512)], in_=ps3)

        # ---- fused pipeline: phase-1 group j woven with attention group j-1 ----
        for j in range(NT):
            if j + 2 < NT:
                fetch_adjm(j + 2)
            ps = ph1_ps.tile([128, 512], F32, tag="p")
            nc.tensor.matmul(ps, w1b, stT[:, ts(j, 512)], start=True, stop=True)
            nc.scalar.activation(h1T[:, ts(j, 512)], ps, AF.Relu, bias=wf["b1"], scale=1.0)
            if j > 0:
                ph2_block(j - 1, 0)
                ph2_block(j - 1, 1)
            ps2 = ph1_ps.tile([128, 512], F32, tag="p")
            nc.tensor.matmul(ps2, w2b, h1T[:, ts(j, 512)], start=True, stop=True)
            nc.scalar.activation(hT[:, ts(j, 512)], ps2, AF.Identity, bias=wf["b2"], scale=1.0)
            if j > 0:
                ph2_block(j - 1, 2)
            ps3 = ph1_ps.tile([128, 512], F32, tag="p")
            nc.tensor.matmul(ps3, wkr, hT[:, ts(j, 512)], start=True, stop=True)
            nc.scalar.copy(out=kTs[:, ts(j, 512)], in_=ps3)
            if j > 0:
                ph2_block(j - 1, 3)
            hx = ph1_ps.tile([128, 512], F32, tag="p")
            for a in range(4):
                nb = j * 4 + a
                nc.tensor.transpose(hx[:, ts(a, 128)], hT[:, ts(nb, 128)].bitcast(F32),
                                    ident)
            nc.scalar.copy(
                out=hblk8[:, 2 * j: 2 * j + 2, :, :],
                in_=hx.rearrange("p (u i e) -> p u i e", u=2, i=2),
            )
        for a in range(4):
            ph2_block(NT - 1, a)

        # aggT / den out of PSUM
        nc.vector.tensor_copy(out=agg_sb[:, 0:512], in_=agg0)
        nc.vector.tensor_copy(out=agg_sb[:, 512:1024], in_=agg1)
        for half, dps in ((0, den0), (1, den1)):
            nc.vector.tensor_copy(out=den_row[0:1, ts(half, 512)], in_=dps)
            nc.vector.tensor_copy(out=den_row_b[0:1, ts(half, 512)], in_=dps)
        ph2_stack.close()

        # ---- head ----
        head_sb = ctx.enter_context(tc.tile_pool(name="head_sb", bufs=3))
        head_ps = ctx.enter_context(tc.tile_pool(name="head_ps", bufs=2, space="PSUM"))
        head_ps2 = ctx.enter_context(tc.tile_pool(name="head_ps2", bufs=1, space="PSUM"))

        for mb in range(MB):
            # h3T_unnorm = wh1^T@aggT + bh1 (x) den   (PSUM-accumulated)
            h3_ps = head_ps.tile([128, 128], F32, tag="hps")
            nc.tensor.matmul(h3_ps, wh1b, agg_sb[:, ts(mb, 128)],
                             start=True, stop=False)
            nc.tensor.matmul(h3_ps, bh1_row_b, den_row_b[0:1, ts(mb, 128)],
                             start=False, stop=True)
            h3 = head_sb.tile([128, 128], BF16, tag="hsb")
            nc.scalar.activation(h3, h3_ps, AF.Relu, bias=0.0, scale=1.0)
            oT_ps = head_ps.tile([8, 128], F32, tag="ops")
            nc.tensor.matmul(oT_ps, wh2b, h3, start=True, stop=True)
            oT = head_sb.tile([8, 128], F32, tag="osb")
            nc.vector.tensor_copy(out=oT, in_=oT_ps)
            # per-m reciprocal of den
            denT_ps = head_ps2.tile([128, 1], F32, tag="dtp")
            nc.tensor.transpose(denT_ps, den_row[0:1, ts(mb, 128)], ident[0:1, 0:1])
            rden = head_sb.tile([128, 1], F32, tag="rdn")
            nc.vector.reciprocal(rden, denT_ps)
            o_ps = head_ps2.tile([128, A], F32, tag="otp")
            nc.tensor.transpose(o_ps, oT, ident[0:8, 0:8])
            o_sb = head_sb.tile([128, A], F32, tag="fin")
            nc.vector.scalar_tensor_tensor(
                out=o_sb, in0=o_ps, scalar=rden, in1=bh2_bcast,
                op0=ALU.mult, op1=ALU.add,
            )
            nc.scalar.dma_start(out=outb[ts(mb, 128), :], in_=o_sb)


# ----------------------------------------------------------------------------
# Host entry point: full inputs in, full output out. 8-way row sharding.
# ----------------------------------------------------------------------------
import numpy as np
import ml_dtypes

N_TOTAL = 8192
N_CORES = 8
M_CORE = N_TOTAL // N_CORES
A8 = 8.0 / np.log(2.0)   # Schraudolph scale for fp8e4m3 bitcast exp

_cached = {}


def _get_nc():
    if "nc" not in _cached:
        _cached["nc"] = build_kernel(n_total=N_TOTAL, m_core=M_CORE)
    return _cached["nc"]


def make_in_maps(state_matrix, adj, w1, b1, w2, b2, wq, wk, wh1, bh1, wh2, bh2):
    state_matrix = np.asarray(state_matrix, dtype=np.float32)
    stateT = np.ascontiguousarray(state_matrix.T).astype(ml_dtypes.bfloat16)
    adj = np.asarray(adj)
    f32 = lambda x: np.ascontiguousarray(np.asarray(x, dtype=np.float32))
    wq_scaled = f32(wq) * np.float32(A8 / np.sqrt(E))
    # Schraudolph mask, transposed: +56 on edges, saturating -32768 elsewhere
    adjm_full = np.where(adj.T > 0, np.int16(56), np.int16(-32768))
    adjm_full = np.ascontiguousarray(adjm_full)
    common = {
        "stateT": stateT,
        "w1": f32(w1), "w2": f32(w2), "wq": wq_scaled, "wk": f32(wk),
        "wh1": f32(wh1), "wh2": f32(wh2),
        "b1": f32(b1).reshape(E, 1), "b2": f32(b2).reshape(E, 1),
        "bh1": f32(bh1).reshape(E, 1), "bh2": f32(bh2).reshape(A, 1),
    }
    in_maps = []
    for c in range(N_CORES):
        rows = slice(c * M_CORE, (c + 1) * M_CORE)
        in_maps.append(
            dict(
                common,
                smbT=np.ascontiguousarray(stateT[:, rows]),
                adjm=np.ascontiguousarray(adjm_full[:, rows]),
            )
        )
    return in_maps


def kernel(state_matrix, adj, w1, b1, w2, b2, wq, wk, wh1, bh1, wh2, bh2):
    from concourse import bass_utils

    in_maps = make_in_maps(
        state_matrix, adj, w1, b1, w2, b2, wq, wk, wh1, bh1, wh2, bh2
    )
    res = bass_utils.run_bass_kernel_spmd(
        _get_nc(), in_maps, core_ids=list(range(N_CORES))
    )
    out = np.concatenate([r["outb"] for r in res.results], axis=0)
    return out.astype(np.float32)


# revision 16
# speedup vs baseline: 2.1350x; 1.0318x over previous
"""CoLightNet Trainium2 Bass kernel (self-contained).

SPMD over 8 cores; core c owns output rows [c*1024, (c+1)*1024).
  inputs : stateT   [S,N]    bf16 (replicated, host-transposed state)
           smbT     [S,Mc]   bf16 (core's own row block, transposed)
           adjm     [N,Mc]   i16  (Schraudolph mask: 56 if edge else -32768)
           w1,w2,wq,wk,wh1 [128,128], wh2 [128,8], biases as [*,1] f32
           (wq is pre-scaled by A8/sqrt(E), A8 = 8/ln2)
  output : outb     [Mc,A]   f32

Math (reference, reformulated):
  h    = relu(state@w1+b1)@w2+b2
  sT   = (h@wk)^T-stationary x (h@wq*A8/sqrt(E))-moving    # A8-scaled scores^T
  wT   = bitcast_fp8e4(int8(round(sT + adjm)))             # Schraudolph exp:
         # edge:   int8(s*A8+56)  bitcast e4m3 ~= exp(s) (+-4%, ratio-cancels)
         # masked: saturates to -128 = -0.0 in fp8         -> exact zero weight
  aggT = h^T(fp8) x wT  via fp8 DoubleRow (2 n-blocks per matmul, 2 rows/cyc)
  den  = ones(fp8) x wT via fp8 DoubleRow
  head : relu(agg/den @ wh1 + bh1) @ wh2 + bh2
       = [relu(agg@wh1 + bh1 (x) den) / den] @ wh2 + bh2   # rank-1 bias trick

Engine budget per 128-node block: Vector 1x STT (mask+exp fused), PE
score 2x216ns + DR agg/den ~4x213ns per block pair, Scalar only phase-1/
head activations + PSUM->SBUF copies. Phase 1 and phase 2 are interleaved
per 512-column group so no engine sits idle behind the MLP chain.
"""

from contextlib import ExitStack

import concourse.bass as bass
import concourse.mybir as mybir
import concourse.tile as tile
from concourse import bacc
from concourse.masks import make_identity

F32 = mybir.dt.float32
F32R = mybir.dt.float32r
BF16 = mybir.dt.bfloat16
FP8 = mybir.dt.float8e4
I8 = mybir.dt.int8
I16 = mybir.dt.int16
AF = mybir.ActivationFunctionType
ALU = mybir.AluOpType
DR = mybir.MatmulPerfMode.DoubleRow

S = 128
E = 128
A = 8


def ts(i, size):
    return slice(i * size, (i + 1) * size)


def build_kernel(n_total=8192, m_core=1024):
    nc = bacc.Bacc("TRN2", debug=False)
    stateT = nc.dram_tensor("stateT", (S, n_total), BF16, kind="ExternalInput").ap()
    smbT = nc.dram_tensor("smbT", (S, m_core), BF16, kind="ExternalInput").ap()
    adjm = nc.dram_tensor("adjm", (n_total, m_core), I16, kind="ExternalInput").ap()
    wt = {}
    for name, shape in [
        ("w1", (S, E)), ("w2", (E, E)), ("wq", (E, E)), ("wk", (E, E)),
        ("wh1", (E, E)), ("wh2", (E, A)),
        ("b1", (E, 1)), ("b2", (E, 1)), ("bh1", (E, 1)), ("bh2", (A, 1)),
    ]:
        wt[name] = nc.dram_tensor(name, shape, F32, kind="ExternalInput").ap()
    outb = nc.dram_tensor("outb", (m_core, A), F32, kind="ExternalOutput").ap()

    with tile.TileContext(nc) as tc:
        colight_body(tc, outb, stateT, smbT, adjm, wt)
    nc.compile()
    return nc


def colight_body(tc, outb, stateT, smbT, adjm, wt):
    nc = tc.nc
    n_total = stateT.shape[1]
    m_core = adjm.shape[1]
    NT = n_total // 512   # n-groups of 512
    NB = n_total // 128   # n-blocks of 128
    NU = NB // 2          # n-block pairs (DoubleRow granule)
    MB = m_core // 128    # m-blocks of 128

    with ExitStack() as ctx:
        singles = ctx.enter_context(tc.tile_pool(name="singles", bufs=1))

        # ---- constants ----
        wf = {}
        for name, shape in [("w1", [S, E]), ("w2", [E, E]), ("wq", [E, E]),
                            ("wk", [E, E]), ("wh1", [E, E]), ("wh2", [E, A]),
                            ("b1", [E, 1]), ("b2", [E, 1]), ("bh1", [E, 1]),
                            ("bh2", [A, 1])]:
            t = singles.tile(shape, F32, tag=f"w_{name}")
            nc.scalar.dma_start(out=t, in_=wt[name])
            wf[name] = t
        w1b = singles.tile([S, E], BF16)
        w2b = singles.tile([E, E], BF16)
        wh1b = singles.tile([E, E], BF16)
        wh2b = singles.tile([E, A], BF16)
        wqr = singles.tile([E, E], F32R)
        wkr = singles.tile([E, E], F32R)
        for dst, src in [(w1b, "w1"), (w2b, "w2"), (wh1b, "wh1"), (wh2b, "wh2"),
                         (wqr, "wq"), (wkr, "wk")]:
            nc.vector.tensor_copy(out=dst, in_=wf[src])
        # bh1 as a 1-partition row (rank-1 bias update), bh2 broadcast tile
        bh1_row = singles.tile([1, E], F32)
        nc.scalar.dma_start(out=bh1_row, in_=wt["bh1"].rearrange("e o -> o e"))
        bh1_row_b = singles.tile([1, E], BF16)
        nc.vector.tensor_copy(out=bh1_row_b, in_=bh1_row)
        bh2_bcast = singles.tile([128, A], F32)
        nc.scalar.dma_start(
            out=bh2_bcast,
            in_=wt["bh2"].rearrange("a o -> o a").to_broadcast([128, A]),
        )
        ident = singles.tile([128, 128], F32)
        make_identity(nc, ident)
        ones8 = singles.tile([128, 1], FP8)
        nc.vector.memset(ones8, 1.0)

        # ---- persistent activations ----
        stT = singles.tile([S, n_total], BF16)
        nc.sync.dma_start(out=stT, in_=stateT)
        smbT_t = singles.tile([S, m_core], BF16)
        nc.sync.dma_start(out=smbT_t, in_=smbT)
        h1T = singles.tile([128, n_total], BF16)
        hT = singles.tile([128, n_total], F32R)
        kTs = singles.tile([E, n_total], F32R)
        qTs = singles.tile([E, m_core], F32R)
        hblk8 = singles.tile([128, NU, 2, E], FP8)  # h, DR-paired [u][i][e]
        agg_sb = singles.tile([128, m_core], BF16)
        den_row = singles.tile([1, m_core], F32)
        den_row_b = singles.tile([1, m_core], BF16)

        ph2_stack = ExitStack()
        ph1_sb = ph2_stack.enter_context(tc.tile_pool(name="ph1_sb", bufs=3))
        ph1_ps = ph2_stack.enter_context(tc.tile_pool(name="ph1_ps", bufs=1, space="PSUM"))
        adj_pool = ph2_stack.enter_context(tc.tile_pool(name="adjm", bufs=4))
        exp_pool = ph2_stack.enter_context(tc.tile_pool(name="expp", bufs=3))
        sc_ps = ph2_stack.enter_context(tc.tile_pool(name="sc_ps", bufs=2, space="PSUM"))
        agg_psp = ph2_stack.enter_context(tc.tile_pool(name="agg_ps", bufs=1, space="PSUM"))

        agg0 = agg_psp.tile([128, 512], F32, tag="agg0")
        agg1 = agg_psp.tile([128, 512], F32, tag="agg1")
        den_ps = agg_psp.tile([64, 512], F32, tag="den")
        den0 = den_ps[0:1, :]
        den1 = den_ps[32:33, :]

        adj_tiles = {}
        def fetch_adjm(ng):
            t = adj_pool.tile([128, 4, m_core], I16)
            for a in range(4):
                nc.sync.dma_start(
                    out=t[:, a, :],
                    in_=adjm[512 * ng + 128 * a: 512 * ng + 128 * (a + 1), :],
                )
            adj_tiles[ng] = t

        fetch_adjm(0)
        fetch_adjm(1)

        exp_tiles = {}
        def ph2_block(ng, a):
            nb = ng * 4 + a
            u, i = nb // 2, nb % 2
            adjt_t = adj_tiles[ng]
            scp = sc_ps.tile([128, m_core], F32, tag="sc")
            nc.tensor.matmul(scp[:, 0:512], kTs[:, ts(nb, 128)], qTs[:, 0:512],
                             start=True, stop=True)
            nc.tensor.matmul(scp[:, 512:1024], kTs[:, ts(nb, 128)], qTs[:, 512:1024],
                             start=True, stop=True)
            if i == 0:
                exp_tiles[u] = exp_pool.tile([128, 2, m_core], I8, tag="e", name=f"ep{u}")
            ep = exp_tiles[u]
            # fused mask + Schraudolph exp: int8(round(s*A8 + {56|-32768}))
            nc.vector.tensor_tensor(out=ep[:, i, :], in0=scp, in1=adjt_t[:, a, :],
                                    op=ALU.add)
            nc.tensor.matmul(den0, ones8, ep[:, i, 0:512].bitcast(FP8),
                             start=(nb == 0), stop=(nb == NB - 1))
            nc.tensor.matmul(den1, ones8, ep[:, i, 512:1024].bitcast(FP8),
                             start=(nb == 0), stop=(nb == NB - 1))
            if i == 1:
                mov0 = ep[:, :, 0:512].bitcast(FP8)
                mov1 = ep[:, :, 512:1024].bitcast(FP8)
                st8 = hblk8[:, u, :, :]
                nc.tensor.matmul(agg0, st8, mov0, start=(u == 0), stop=(u == NU - 1),
                                 perf_mode=DR)
                nc.tensor.matmul(agg1, st8, mov1, start=(u == 0), stop=(u == NU - 1),
                                 perf_mode=DR)
                del exp_tiles[u]
                if a == 3:
                    del adj_tiles[ng]

        # ---- q-path, stage-major across its two column groups (phase 2
        #      needs qTs for every block; sc_ps is idle here, borrow it) ----
        qp_ps = [sc_ps.tile([128, m_core], F32, tag="sc", name=f"qp{j}")
                 for j in range(2)]
        for j in range(2):
            nc.tensor.matmul(qp_ps[j][:, 0:512], w1b, smbT_t[:, ts(j, 512)],
                             start=True, stop=True)
        h1m = [ph1_sb.tile([128, 512], BF16, tag="sbq", name=f"h1m{j}")
               for j in range(2)]
        for j in range(2):
            nc.scalar.activation(h1m[j], qp_ps[j][:, 0:512], AF.Relu,
                                 bias=wf["b1"], scale=1.0)
        for j in range(2):
            nc.tensor.matmul(qp_ps[j][:, 512:1024], w2b, h1m[j], start=True, stop=True)
        hm = [ph1_sb.tile([128, 512], F32R, tag="sbq2", name=f"hm{j}")
              for j in range(2)]
        for j in range(2):
            nc.scalar.activation(hm[j], qp_ps[j][:, 512:1024], AF.Identity,
                                 bias=wf["b2"], scale=1.0)
        for j in range(2):
            nc.tensor.matmul(qp_ps[j][:, 0:512], wqr, hm[j], start=True, stop=True)
        for j in range(2):
            nc.scalar.copy(out=qTs[:, ts(j, 512)], in_=qp_ps[j][:, 0:512])

        # ---- fused pipeline: phase-1 group j woven with attention group j-1 ----
        for j in range(NT):
            if j + 2 < NT:
                fetch_adjm(j + 2)
            ps = ph1_ps.tile([128, 512], F32, tag="p")
            nc.tensor.matmul(ps, w1b, stT[:, ts(j, 512)], start=True, stop=True)
            nc.scalar.activation(h1T[:, ts(j, 512)], ps, AF.Relu, bias=wf["b1"], scale=1.0)
            if j > 0:
                ph2_block(j - 1, 0)
                ph2_block(j - 1, 1)
            ps2 = ph1_ps.tile([128, 512], F32, tag="p")
            nc.tensor.matmul(ps2, w2b, h1T[:, ts(j, 512)], start=True, stop=True)
            nc.scalar.activation(hT[:, ts(j, 512)], ps2, AF.Identity, bias=wf["b2"], scale=1.0)
            if j > 0:
                ph2_block(j - 1, 2)
            ps3 = ph1_ps.tile([128, 512], F32, tag="p")
            nc.tensor.matmul(ps3, wkr, hT[:, ts(j, 512)], start=True, stop=True)
            nc.scalar.copy(out=kTs[:, ts(j, 512)], in_=ps3)
            if j > 0:
                ph2_block(j - 1, 3)
            hx = ph1_ps.tile([128, 512], F32, tag="p")
            for a in range(4):
                nb = j * 4 + a
                nc.tensor.transpose(hx[:, ts(a, 128)], hT[:, ts(nb, 128)].bitcast(F32),
                                    ident)
            nc.scalar.copy(
                out=hblk8[:, 2 * j: 2 * j + 2, :, :],
                in_=hx.rearrange("p (u i e) -> p u i e", u=2, i=2),
            )
        for a in range(4):
            ph2_block(NT - 1, a)

        # aggT / den out of PSUM
        nc.vector.tensor_copy(out=agg_sb[:, 0:512], in_=agg0)
        nc.vector.tensor_copy(out=agg_sb[:, 512:1024], in_=agg1)
        for half, dps in ((0, den0), (1, den1)):
            nc.vector.tensor_copy(out=den_row[0:1, ts(half, 512)], in_=dps[0:1, :])
            nc.vector.tensor_copy(out=den_row_b[0:1, ts(half, 512)], in_=dps[0:1, :])
        ph2_stack.close()

        # ---- head ----
        head_sb = ctx.enter_context(tc.tile_pool(name="head_sb", bufs=3))
        head_ps = ctx.enter_context(tc.tile_pool(name="head_ps", bufs=2, space="PSUM"))
        head_ps2 = ctx.enter_context(tc.tile_pool(name="head_ps2", bufs=1, space="PSUM"))

        for mb in range(MB):
            # h3T_unnorm = wh1^T@aggT + bh1 (x) den   (PSUM-accumulated)
            h3_ps = head_ps.tile([128, 128], F32, tag="hps")
            nc.tensor.matmul(h3_ps, wh1b, agg_sb[:, ts(mb, 128)],
                             start=True, stop=False)
            nc.tensor.matmul(h3_ps, bh1_row_b, den_row_b[0:1, ts(mb, 128)],
                             start=False, stop=True)
            h3 = head_sb.tile([128, 128], BF16, tag="hsb")
            nc.scalar.activation(h3, h3_ps, AF.Relu, bias=0.0, scale=1.0)
            oT_ps = head_ps.tile([8, 128], F32, tag="ops")
            nc.tensor.matmul(oT_ps, wh2b, h3, start=True, stop=True)
            oT = head_sb.tile([8, 128], F32, tag="osb")
            nc.vector.tensor_copy(out=oT, in_=oT_ps)
            # per-m reciprocal of den
            denT_ps = head_ps2.tile([128, 1], F32, tag="dtp")
            nc.tensor.transpose(denT_ps, den_row[0:1, ts(mb, 128)], ident[0:1, 0:1])
            rden = head_sb.tile([128, 1], F32, tag="rdn")
            nc.vector.reciprocal(rden, denT_ps)
            o_ps = head_ps2.tile([128, A], F32, tag="otp")
            nc.tensor.transpose(o_ps, oT, ident[0:8, 0:8])
            o_sb = head_sb.tile([128, A], F32, tag="fin")
            nc.vector.scalar_tensor_tensor(
                out=o_sb, in0=o_ps, scalar=rden, in1=bh2_bcast,
                op0=ALU.mult, op1=ALU.add,
            )
            nc.scalar.dma_start(out=outb[ts(mb, 128), :], in_=o_sb)


# ----------------------------------------------------------------------------
# Host entry point: full inputs in, full output out. 8-way row sharding.
# ----------------------------------------------------------------------------
import numpy as np
import ml_dtypes

N_TOTAL = 8192
N_CORES = 8
M_CORE = N_TOTAL // N_CORES
A8 = 8.0 / np.log(2.0)   # Schraudolph scale for fp8e4m3 bitcast exp

_cached = {}


def _get_nc():
    if "nc" not in _cached:
        _cached["nc"] = build_kernel(n_total=N_TOTAL, m_core=M_CORE)
    return _cached["nc"]


def make_in_maps(state_matrix, adj, w1, b1, w2, b2, wq, wk, wh1, bh1, wh2, bh2):
    state_matrix = np.asarray(state_matrix, dtype=np.float32)
    stateT = np.ascontiguousarray(state_matrix.T).astype(ml_dtypes.bfloat16)
    adj = np.asarray(adj)
    f32 = lambda x: np.ascontiguousarray(np.asarray(x, dtype=np.float32))
    wq_scaled = f32(wq) * np.float32(A8 / np.sqrt(E))
    # Schraudolph mask, transposed: +56 on edges, saturating -32768 elsewhere
    adjm_full = np.where(adj.T > 0, np.int16(56), np.int16(-32768))
    adjm_full = np.ascontiguousarray(adjm_full)
    common = {
        "stateT": stateT,
        "w1": f32(w1), "w2": f32(w2), "wq": wq_scaled, "wk": f32(wk),
        "wh1": f32(wh1), "wh2": f32(wh2),
        "b1": f32(b1).reshape(E, 1), "b2": f32(b2).reshape(E, 1),
        "bh1": f32(bh1).reshape(E, 1), "bh2": f32(bh2).reshape(A, 1),
    }
    in_maps = []
    for c in range(N_CORES):
        rows = slice(c * M_CORE, (c + 1) * M_CORE)
        in_maps.append(
            dict(
                common,
                smbT=np.ascontiguousarray(stateT[:, rows]),
                adjm=np.ascontiguousarray(adjm_full[:, rows]),
            )
        )
    return in_maps


def kernel(state_matrix, adj, w1, b1, w2, b2, wq, wk, wh1, bh1, wh2, bh2):
    from concourse import bass_utils

    in_maps = make_in_maps(
        state_matrix, adj, w1, b1, w2, b2, wq, wk, wh1, bh1, wh2, bh2
    )
    res = bass_utils.run_bass_kernel_spmd(
        _get_nc(), in_maps, core_ids=list(range(N_CORES))
    )
    out = np.concatenate([r["outb"] for r in res.results], axis=0)
    return out.astype(np.float32)
